# revision 3
# baseline (speedup 1.0000x reference)
"""CorrFast correlation kernel for Trainium2 (8 NeuronCores).

out[b, o, h, w], o = 21*di+dj over even displacements (2*di-20, 2*dj-20);
the final (B, 441, H, W) output is the o-major reinterpretation of the
pixel-major (b, h, w, o) array (matches the reference's transpose+reshape).

Strategy:
  - Shard (batch=4) x (H halves) -> 8 cores.
  - All displacements are even, so the problem splits into 4 parity classes
    (h%2, w%2). Per class, pixels tile into blocks of 8(rows) x 16(cols).
  - Host pre-pads feat2, pre-splits both feats into parity classes, packs
    f1 per-block ([96, 80*128] bf16) and f2 per-(class, xb) windows
    ([96, 4*5*52*36] bf16) so every matmul operand is a contiguous slice.
  - Per block: 2 matmuls (K=96 channels, M=128 pixels, N=504) stream the
    f2 source window -> PSUM band [128, 1008] (bf16 in, fp32 accumulate).
  - ACT/DVE evict PSUM->SBUF, DMA stores the raw band to DRAM.
  - Host extracts the 441-offset diagonal band per pixel via a strided view
    (band col (g+di)*36 + (x+dj)) and assembles the output.
"""

import numpy as np
import sys

if "/opt/trn_rl_repo" not in sys.path:
    sys.path.insert(0, "/opt/trn_rl_repo")

import ml_dtypes

BF16 = ml_dtypes.bfloat16

B, C, H, W = 4, 96, 128, 160
D_PAD = 20
NOFF = 21          # offsets per axis
O = NOFF * NOFF    # 441
N_CORES = 8
HH = H // 2        # 64 rows per core
F2H = HH + 2 * D_PAD   # 104
F2W = W + 2 * D_PAD    # 200

# per-class geometry (class grid is 32 x 80 per core)
GB, XB = 4, 5          # block grid
G, X = 8, 16           # block = 8 class-rows x 16 class-cols = 128 pixels
NR, NJ = G + NOFF - 1, X + NOFF - 1   # 28 source rows, 36 source cols
NCLS = 4
NBLK = NCLS * GB * XB  # 80 blocks per core
BAND = NR * NJ         # 1008 band columns
CLS_ROWS = 2 * GB * G // 2 + NOFF - 1  # 52 source class-rows per class

F1_FLAT = NBLK * 128            # 10240
F2_WIN = CLS_ROWS * NJ          # 52*36 = 1872 per (cls, xb)
F2_FLAT = NCLS * XB * F2_WIN    # 37440

BAND_BF16 = False

_cache = {}


def _build_program():
    import concourse.tile as tile
    from concourse import bacc, mybir

    band_dt = mybir.dt.bfloat16 if BAND_BF16 else mybir.dt.float32

    nc = bacc.Bacc("TRN2", target_bir_lowering=False, debug=False,
                   num_devices=N_CORES)

    f1_d = nc.dram_tensor("f1b", [C, F1_FLAT], mybir.dt.bfloat16,
                          kind="ExternalInput").ap()
    f2_d = nc.dram_tensor("f2b", [C, F2_FLAT], mybir.dt.bfloat16,
                          kind="ExternalInput").ap()
    band_d = nc.dram_tensor("band", [NBLK, 128, BAND], band_dt,
                            kind="ExternalOutput").ap()

    with tile.TileContext(nc) as tc:
        with (
            tc.tile_pool(name="feat", bufs=1) as feat_pool,
            tc.tile_pool(name="band", bufs=4) as band_pool,
            tc.tile_pool(name="ps", bufs=4, space="PSUM") as psum_pool,
        ):
            f1_sb = feat_pool.tile([C, F1_FLAT], mybir.dt.bfloat16)
            f2_sb = feat_pool.tile([C, F2_FLAT], mybir.dt.bfloat16)
            nc.sync.dma_start(f1_sb[:], f1_d[:])
            nc.sync.dma_start(f2_sb[:], f2_d[:])

            blk = 0
            for cls in range(NCLS):
                for gb in range(GB):
                    for xb in range(XB):
                        lhsT = f1_sb[:, blk * 128:(blk + 1) * 128]
                        base = (cls * XB + xb) * F2_WIN + gb * G * NJ
                        ps = psum_pool.tile([128, 1024], mybir.dt.float32)
                        nc.tensor.matmul(ps[:, 0:504], lhsT,
                                         f2_sb[:, base:base + 504])
                        nc.tensor.matmul(ps[:, 512:1016], lhsT,
                                         f2_sb[:, base + 504:base + 1008])
                        bd = band_pool.tile([128, BAND], band_dt)
                        nc.scalar.copy(bd[:, 0:504], ps[:, 0:504])
                        nc.vector.tensor_copy(bd[:, 504:1008],
                                              ps[:, 512:1016])
                        nc.sync.dma_start(band_d[blk], bd[:])
                        blk += 1

    nc.compile()
    return nc


def _get_nc():
    if "nc" not in _cache:
        _cache["nc"] = _build_program()
    return _cache["nc"]


def _pack_inputs(feat1, feat2):
    """Host-side shard + parity split + block packing -> per-core maps."""
    f2p = np.pad(feat2, ((0, 0), (0, 0), (D_PAD, D_PAD), (D_PAD, D_PAD)))
    f1_bf = feat1.astype(BF16)
    f2_bf = f2p.astype(BF16)
    in_maps = []
    for core in range(N_CORES):
        b, half = core // 2, core % 2
        h0 = half * HH
        f1c = f1_bf[b, :, h0:h0 + HH, :]        # (96, 64, 160)
        f2c = f2_bf[b, :, h0:h0 + F2H, :]       # (96, 104, 200)

        f1b = np.empty((C, NCLS, GB, XB, G, X), dtype=BF16)
        f2b = np.empty((C, NCLS, XB, CLS_ROWS, NJ), dtype=BF16)
        for ph in range(2):
            for pw in range(2):
                cls = ph * 2 + pw
                c1 = f1c[:, ph::2, pw::2]       # (96, 32, 80)
                f1b[:, cls] = c1.reshape(C, GB, G, XB, X).transpose(
                    0, 1, 3, 2, 4)
                c2 = f2c[:, ph::2, pw::2]       # (96, 52, 100)
                for xb in range(XB):
                    f2b[:, cls, xb] = c2[:, :, 16 * xb:16 * xb + NJ]
        in_maps.append({
            "f1b": np.ascontiguousarray(f1b.reshape(C, F1_FLAT)),
            "f2b": np.ascontiguousarray(f2b.reshape(C, F2_FLAT)),
        })
    return in_maps


def _extract(band_core):
    """band_core: (NBLK,128,1008) -> (2,2,32,80,441) per-class correlation."""
    v = band_core.reshape(2, 2, GB, XB, G, X, BAND)
    s = v.strides
    # diag[ph,pw,gb,xb,g,x,di,dj] = v[ph,pw,gb,xb,g,x,(g+di)*NJ+(x+dj)]
    diag = np.lib.stride_tricks.as_strided(
        v,
        shape=(2, 2, GB, XB, G, X, NOFF, NOFF),
        strides=(s[0], s[1], s[2], s[3],
                 s[4] + NJ * s[6], s[5] + s[6],
                 NJ * s[6], s[6]),
    )
    out = np.ascontiguousarray(
        np.transpose(diag, (0, 1, 2, 4, 3, 5, 6, 7))).reshape(
        2, 2, GB * G, XB * X, O)
    return out


def kernel(feat1: np.ndarray, feat2: np.ndarray) -> np.ndarray:
    from concourse.bass_utils import run_bass_kernel_spmd

    nc = _get_nc()
    in_maps = _pack_inputs(np.asarray(feat1), np.asarray(feat2))
    res = run_bass_kernel_spmd(nc, in_maps, list(range(N_CORES)))

    out_bhwo = np.empty((B, H, W, O), dtype=np.float32)
    for core in range(N_CORES):
        b, half = core // 2, core % 2
        h0 = half * HH
        band = res.results[core]["band"]
        if BAND_BF16:
            band = band.view(BF16) if band.dtype != BF16 else band
        cls = _extract(band).astype(np.float32, copy=False)
        for ph in range(2):
            for pw in range(2):
                out_bhwo[b, h0 + ph:h0 + HH:2, pw:W:2, :] = cls[ph, pw]
    return out_bhwo.reshape(B, O, H, W)


if __name__ == "__main__":
    rng = np.random.default_rng(0)
    a = rng.standard_normal((B, C, H, W)).astype(np.float32)
    bb = rng.standard_normal((B, C, H, W)).astype(np.float32)
    out = kernel(a, bb)
    print("out shape:", out.shape, out.dtype)


# revision 19
# speedup vs baseline: 1.6148x; 1.6148x over previous
"""CorrFast correlation kernel for Trainium2 (8 NeuronCores).

out[b, o, h, w], o = 21*di+dj over even displacements (2*di-20, 2*dj-20);
the final (B, 441, H, W) output is the o-major reinterpretation of the
pixel-major (b, h, w, o) array (matches the reference's transpose+reshape).

Strategy:
  - Shard (batch=4) x (H halves) -> 8 cores.
  - All displacements are even, so the problem splits into 4 parity classes
    (h%2, w%2). Per class, pixels tile into blocks of 8(rows) x 16(cols).
  - Host pre-pads feat2, pre-splits both feats into parity classes, packs
    f1 per-block ([96, 80*128] bf16) and f2 per-(class, xb) windows
    ([96, 4*5*52*36] bf16) so every matmul operand is a contiguous slice.
  - Per block: 2 matmuls (K=96 channels, M=128 pixels, N=504) stream the
    f2 source window -> PSUM band [128, 1008] (bf16 in, fp32 accumulate).
  - ACT/DVE evict PSUM->SBUF (casting to bf16), DMA stores the raw band.
  - Host extracts the 441-offset diagonal band per pixel via a strided view
    (band col (g+di)*36 + (x+dj)) and assembles the output.
"""

import numpy as np
import sys

if "/opt/trn_rl_repo" not in sys.path:
    sys.path.insert(0, "/opt/trn_rl_repo")

import ml_dtypes

BF16 = ml_dtypes.bfloat16

B, C, H, W = 4, 96, 128, 160
D_PAD = 20
NOFF = 21          # offsets per axis
O = NOFF * NOFF    # 441
N_CORES = 8
HH = H // 2        # 64 rows per core
F2H = HH + 2 * D_PAD   # 104
F2W = W + 2 * D_PAD    # 200

# per-class geometry (class grid is 32 x 80 per core)
GB, XB = 4, 5          # block grid
G, X = 8, 16           # block = 8 class-rows x 16 class-cols = 128 pixels
NR, NJ = G + NOFF - 1, X + NOFF - 1   # 28 source rows, 36 source cols
NCLS = 4
NBLK = NCLS * GB * XB  # 80 blocks per core
BAND = NR * NJ         # 1008 band columns
CLS_ROWS = GB * G + NOFF - 1  # 52 source class-rows per class

F1_CLS = GB * XB * 128          # 2560 per class
F1_FLAT = NCLS * F1_CLS         # 10240
F2_WIN = CLS_ROWS * NJ          # 52*36 = 1872 per (cls, xb)
F2_CLS = XB * F2_WIN            # 9360 per class
F2_FLAT = NCLS * F2_CLS         # 37440

BAND_BF16 = True

_cache = {}


def _emit(nc, tc, ctx, f1_d, f2_d, band_d, band_dt, repeat=0,
          do_mm=True, do_evict=True, do_store=True, loads_in_body=False):
    """Emit the kernel body. repeat>0 wraps block loop in For_i (benching)."""
    from concourse import mybir

    feat_pool = ctx.enter_context(tc.tile_pool(name="feat", bufs=1))
    band_pool = ctx.enter_context(tc.tile_pool(name="band", bufs=8))
    psum_pool = ctx.enter_context(tc.tile_pool(name="ps", bufs=4,
                                               space="PSUM"))

    # one tile per class so matmuls start as soon as their class is loaded
    f1_sb, f2_sb = [], []
    for cls in range(NCLS):
        t1 = feat_pool.tile([C, F1_CLS], mybir.dt.bfloat16, tag=f"f1_{cls}")
        t2 = feat_pool.tile([C, XB, CLS_ROWS, NJ], mybir.dt.bfloat16,
                            tag=f"f2_{cls}")
        f1_sb.append(t1)
        f2_sb.append(t2)

    def loads():
        # SWDGE ring: keeps both HWDGE rings free for band stores
        for cls in range(NCLS):
            nc.gpsimd.dma_start(f1_sb[cls][:],
                                f1_d[:, cls * F1_CLS:(cls + 1) * F1_CLS])
            nc.gpsimd.dma_start(
                f2_sb[cls][:],
                f2_d[:, cls * F2_CLS:(cls + 1) * F2_CLS].rearrange(
                    "c (a r j) -> c a r j", a=XB, r=CLS_ROWS))

    if not loads_in_body:
        loads()

    def body():
        if loads_in_body:
            loads()
        blk = 0
        for cls in range(NCLS):
            for gb in range(GB):
                for xb in range(XB):
                    i1 = (gb * XB + xb) * 128
                    lhsT = f1_sb[cls][:, i1:i1 + 128]
                    f2flat = f2_sb[cls].rearrange("c a r j -> c (a r j)")
                    base = xb * F2_WIN + gb * G * NJ
                    ps = psum_pool.tile([128, 1024], mybir.dt.float32)
                    if do_mm:
                        nc.tensor.matmul(ps[:, 0:504], lhsT,
                                         f2flat[:, base:base + 504])
                        nc.tensor.matmul(ps[:, 512:1016], lhsT,
                                         f2flat[:, base + 504:base + 1008])
                    bd = band_pool.tile([128, BAND], band_dt)
                    if do_evict:
                        nc.scalar.copy(bd[:, 0:504], ps[:, 0:504])
                        nc.vector.tensor_copy(bd[:, 504:1008],
                                              ps[:, 512:1016])
                    if do_store:
                        eng = nc.sync if blk % 2 == 0 else nc.scalar
                        eng.dma_start(band_d[blk], bd[:])
                    blk += 1

    if repeat:
        with tc.For_i(0, repeat, 1):
            body()
    else:
        body()


def _build_program():
    import concourse.tile as tile
    from contextlib import ExitStack
    from concourse import bacc, mybir

    band_dt = mybir.dt.bfloat16 if BAND_BF16 else mybir.dt.float32
    nc = bacc.Bacc("TRN2", target_bir_lowering=False, debug=False,
                   num_devices=N_CORES)
    f1_d = nc.dram_tensor("f1b", [C, F1_FLAT], mybir.dt.bfloat16,
                          kind="ExternalInput").ap()
    f2_d = nc.dram_tensor("f2b", [C, F2_FLAT], mybir.dt.bfloat16,
                          kind="ExternalInput").ap()
    band_d = nc.dram_tensor("band", [NBLK, 128, BAND], band_dt,
                            kind="ExternalOutput").ap()
    with tile.TileContext(nc) as tc:
        with ExitStack() as ctx:
            _emit(nc, tc, ctx, f1_d, f2_d, band_d, band_dt)
    nc.compile()
    return nc


def _get_nc():
    if "nc" not in _cache:
        _cache["nc"] = _build_program()
    return _cache["nc"]


def _pack_inputs(feat1, feat2):
    """Host-side shard + parity split + block packing -> per-core maps."""
    f2p = np.pad(feat2, ((0, 0), (0, 0), (D_PAD, D_PAD), (D_PAD, D_PAD)))
    f1_bf = feat1.astype(BF16)
    f2_bf = f2p.astype(BF16)
    in_maps = []
    for core in range(N_CORES):
        b, half = core // 2, core % 2
        h0 = half * HH
        f1c = f1_bf[b, :, h0:h0 + HH, :]        # (96, 64, 160)
        f2c = f2_bf[b, :, h0:h0 + F2H, :]       # (96, 104, 200)

        f1b = np.empty((C, NCLS, GB, XB, G, X), dtype=BF16)
        f2b = np.empty((C, NCLS, XB, CLS_ROWS, NJ), dtype=BF16)
        for ph in range(2):
            for pw in range(2):
                cls = ph * 2 + pw
                c1 = f1c[:, ph::2, pw::2]       # (96, 32, 80)
                f1b[:, cls] = c1.reshape(C, GB, G, XB, X).transpose(
                    0, 1, 3, 2, 4)
                c2 = f2c[:, ph::2, pw::2]       # (96, 52, 100)
                for xb in range(XB):
                    f2b[:, cls, xb] = c2[:, :, 16 * xb:16 * xb + NJ]
        in_maps.append({
            "f1b": np.ascontiguousarray(f1b.reshape(C, F1_FLAT)),
            "f2b": np.ascontiguousarray(f2b.reshape(C, F2_FLAT)),
        })
    return in_maps


def _extract(band_core):
    """band_core: (NBLK,128,1008) -> (2,2,32,80,441) per-class correlation."""
    v = band_core.reshape(2, 2, GB, XB, G, X, BAND)
    s = v.strides
    # diag[ph,pw,gb,xb,g,x,di,dj] = v[ph,pw,gb,xb,g,x,(g+di)*NJ+(x+dj)]
    diag = np.lib.stride_tricks.as_strided(
        v,
        shape=(2, 2, GB, XB, G, X, NOFF, NOFF),
        strides=(s[0], s[1], s[2], s[3],
                 s[4] + NJ * s[6], s[5] + s[6],
                 NJ * s[6], s[6]),
    )
    out = np.ascontiguousarray(
        np.transpose(diag, (0, 1, 2, 4, 3, 5, 6, 7))).reshape(
        2, 2, GB * G, XB * X, O)
    return out


def kernel(feat1: np.ndarray, feat2: np.ndarray) -> np.ndarray:
    from concourse.bass_utils import run_bass_kernel_spmd

    nc = _get_nc()
    in_maps = _pack_inputs(np.asarray(feat1), np.asarray(feat2))
    res = run_bass_kernel_spmd(nc, in_maps, list(range(N_CORES)))

    out_bhwo = np.empty((B, H, W, O), dtype=np.float32)
    for core in range(N_CORES):
        b, half = core // 2, core % 2
        h0 = half * HH
        band = res.results[core]["band"]
        if band.dtype != np.float32:
            band = band.view(BF16) if band.dtype.itemsize == 2 else band
        cls = _extract(band).astype(np.float32, copy=False)
        for ph in range(2):
            for pw in range(2):
                out_bhwo[b, h0 + ph:h0 + HH:2, pw:W:2, :] = cls[ph, pw]
    return out_bhwo.reshape(B, O, H, W)


if __name__ == "__main__":
    rng = np.random.default_rng(0)
    a = rng.standard_normal((B, C, H, W)).astype(np.float32)
    bb = rng.standard_normal((B, C, H, W)).astype(np.float32)
    out = kernel(a, bb)
    print("out shape:", out.shape, out.dtype)
